# revision 34
# baseline (speedup 1.0000x reference)
"""AnchorAttention Trainium2 kernel (8 NeuronCores, SPMD, no collectives).

Math (per batch): gather anchor rows of hidden_states, LayerNorm, QKV
projections, dense attention among anchors only, out-projection, scatter
back (non-anchor rows of the output are zero; keys are anchors only).

Sharding: core c handles batch c//2 and HEAD GROUP c%2 (4 of 8 heads).
Both cores of a pair see the same gathered anchor tokens; each computes
q/k/v and attention for its 4 heads over ALL anchors, then a partial
out-projection (sum over its heads). The host adds the two partials
(+ output bias) — out-projection is linear in heads, so no collective
is needed.

Device layout (contraction dims on partitions):
  zT   per 512-token chunk: (128, 6, 512)  z = (x-mu)*rstd, d on partitions
  qT   (128, 4, NA)  per head 128 rows: 96 hd + row96 == 1.0 (mask helper)
  kT   (128, 4, NA)  per head 128 rows: 96 hd + row96 == key-pad mask
  v    (128, T, 4, 128) plain layout: 96 head dims + ones col + zero pad
  scores^T (tk, tq) per (head, tk-tile, query-half); probs = exp(scale*s)
  avT  (128, NQH) accumulated over tk; row 96 = softmax denominator
  outT (768, NA) = sum_h Wo_h^T @ (avT_h / denom_h)   [bias added on host]

LayerNorm's affine (ln_g, ln_b) is folded into the weights on the host:
W~ = W * g, bias~ = W @ b + bias.
"""

import numpy as np
import ml_dtypes

import concourse.bass as bass
import concourse.mybir as mybir
import concourse.tile as tile
from concourse import bacc
from concourse.bass_utils import run_bass_kernel_spmd

BF16 = ml_dtypes.bfloat16
F32 = mybir.dt.float32
BF = mybir.dt.bfloat16

B, S, D, H, HD = 4, 2048, 768, 8, 96
HL = H // 2           # heads per core
J = D // 128          # contraction blocks
EPS = 1e-5
SCALE = 1.0 / np.sqrt(HD)
MASK_NEG = -30000.0   # exp(SCALE * (qk + MASK_NEG)) == 0 in fp32


def _chunks(total, step):
    out = []
    c = 0
    while c < total:
        out.append((c, min(step, total - c)))
        c += step
    return out


def build(NA, QC):
    """Build the per-core Bacc graph for padded anchor count NA."""
    assert NA % 128 == 0 and QC % 64 == 0 and QC <= NA
    T = NA // 128
    QSPLIT = _chunks(QC, 576)      # attention query units (<= 576 wide)
    CH = _chunks(NA, 512)          # token chunks (512-wide except tail)

    nc = bacc.Bacc("TRN2", target_bir_lowering=False, debug=False, num_devices=8)

    x_ext = nc.dram_tensor("x", [NA, D], BF, kind="ExternalInput").ap()
    wq_ext = nc.dram_tensor("wq", [128, J * HL * 128], BF, kind="ExternalInput").ap()
    wk_ext = nc.dram_tensor("wk", [128, J * HL * 128], BF, kind="ExternalInput").ap()
    wv_ext = nc.dram_tensor("wv", [128, J * HL * 96], BF, kind="ExternalInput").ap()
    wo_ext = nc.dram_tensor("wo", [128, HL * D], BF, kind="ExternalInput").ap()
    bq_ext = nc.dram_tensor("bq", [128, HL], F32, kind="ExternalInput").ap()
    bk_ext = nc.dram_tensor("bk", [128, HL], F32, kind="ExternalInput").ap()
    bv_ext = nc.dram_tensor("bv", [HL * 96], F32, kind="ExternalInput").ap()
    km_ext = nc.dram_tensor("km", [1, NA], BF, kind="ExternalInput").ap()
    out_ext = nc.dram_tensor("out", [D, QC], BF, kind="ExternalOutput").ap()

    with tile.TileContext(nc) as tc:
        with (
            tc.tile_pool(name="singles", bufs=1) as singles,
            tc.tile_pool(name="work", bufs=5) as work,
            tc.tile_pool(name="probs", bufs=20) as probs_pool,
        ):
            # ---- x tiles first (LN needs them immediately; queues are FIFO
            # so anything emitted before them would delay the whole prologue)
            x_tiles = []
            for i in range(T):
                x_i = work.tile([128, D], BF, tag="x", bufs=T)
                nc.sync.dma_start(out=x_i, in_=x_ext[i * 128:(i + 1) * 128, :])
                x_tiles.append(x_i)

            # ---- weights / constants into SBUF (contiguous per partition).
            wq_sb = singles.tile([128, J, HL * 128], BF)
            wk_sb = singles.tile([128, J, HL * 128], BF)
            wv_sb = singles.tile([128, J, HL * 96], BF)
            wo_sb = singles.tile([128, HL, D], BF)
            wq_v = wq_ext.rearrange("p (j e) -> p j e", j=J)
            wk_v = wk_ext.rearrange("p (j e) -> p j e", j=J)
            wv_v = wv_ext.rearrange("p (j e) -> p j e", j=J)
            for j in range(J):
                nc.sync.dma_start(out=wk_sb[:, j, :], in_=wk_v[:, j, :])
                nc.sync.dma_start(out=wq_sb[:, j, :], in_=wq_v[:, j, :])
                nc.sync.dma_start(out=wv_sb[:, j, :], in_=wv_v[:, j, :])
            nc.sync.dma_start(out=wo_sb, in_=wo_ext)
            bq_sb = singles.tile([128, HL], F32)
            nc.sync.dma_start(out=bq_sb, in_=bq_ext)
            bk_sb = singles.tile([128, HL], F32)
            nc.sync.dma_start(out=bk_sb, in_=bk_ext)
            bv_sb = singles.tile([128, HL * 96], F32)
            bv_bcast = bass.AP(
                tensor=bv_ext.tensor, offset=bv_ext.offset,
                ap=[[0, 128], [1, HL * 96]],
            )
            nc.gpsimd.dma_start(out=bv_sb, in_=bv_bcast)

            ones96 = singles.tile([1, 96], BF)
            nc.vector.memset(ones96, 1.0)
            eps_sb = singles.tile([128, 1], F32)
            nc.vector.memset(eps_sb, EPS)
            ident = singles.tile([128, 128], BF)
            from concourse.masks import make_identity
            make_identity(nc, ident)

            zT = [singles.tile([128, J, cw], BF, name=f"zT{c}")
                  for c, (c0, cw) in enumerate(CH)]

            def zt_slice(j, c0, cw):
                ci = c0 // 512
                off = c0 % 512
                assert off + cw <= CH[ci][1]
                return zT[ci][:, j, off:off + cw]

            kT = singles.tile([128, HL, NA], BF)
            qT = singles.tile([128, HL, QC], BF)
            v_sb = singles.tile([128, T, HL, 128], BF)
            avn = singles.tile([128, HL, QC], BF)
            nc.gpsimd.memset(avn[96:128, :, :], 0.0)

            # v columns: 0..95 head dims, 96 ones (denominator), 97.. zero
            # (padding to 128 weights keeps FWL on for the av matmuls)
            nc.vector.memset(v_sb[:, :, :, 96:97], 1.0)
            nc.gpsimd.memset(v_sb[:, :, :, 97:128], 0.0)

            with (
                tc.tile_pool(name="ps_proj", bufs=2, space="PSUM") as ps_proj,
                tc.tile_pool(name="ps_t", bufs=3, space="PSUM") as ps_t,
            ):
                # Pipeline per 512-token chunk: LN/z -> transpose (on the
                # otherwise-idle PE) -> K/Q/V projections for that chunk.
                for ci, (c0, cw) in enumerate(CH):
                    tlo, thi = c0 // 128, (c0 + cw) // 128
                    for i in range(tlo, thi):
                        x_i = x_tiles[i]
                        x_g = x_i.rearrange("p (n f) -> p n f", f=384)
                        stats = work.tile([128, 2, 6], F32, tag="stats")
                        for g in range(2):
                            nc.vector.bn_stats(out=stats[:, g, :], in_=x_g[:, g, :])
                        mv = work.tile([128, 2], F32, tag="mv")
                        nc.vector.bn_aggr(out=mv, in_=stats)
                        sd = work.tile([128, 1], F32, tag="sd")
                        nc.scalar.activation(
                            out=sd, in_=mv[:, 1:2],
                            func=mybir.ActivationFunctionType.Sqrt,
                            bias=eps_sb, scale=1.0,
                        )
                        rstd = work.tile([128, 1], F32, tag="rstd")
                        nc.vector.reciprocal(out=rstd, in_=sd)
                        z_i = work.tile([128, D], BF, tag="z")
                        nc.vector.tensor_scalar(
                            out=z_i, in0=x_i,
                            scalar1=mv[:, 0:1], scalar2=rstd,
                            op0=mybir.AluOpType.subtract, op1=mybir.AluOpType.mult,
                        )
                        ioff = (i - tlo) * 128
                        for j in range(J):
                            tp = ps_t.tile([128, 128], BF, tag="tp")
                            nc.tensor.transpose(
                                tp, z_i[:, j * 128:(j + 1) * 128], ident)
                            nc.scalar.activation(
                                out=zT[ci][:, j, ioff:ioff + 128], in_=tp,
                                func=mybir.ActivationFunctionType.Copy,
                            )

                    # K / Q projections for this chunk (local heads).
                    # Q only covers [0, QC) — queries past the last anchor
                    # are never read.
                    qw_c = min(cw, max(0, QC - c0))
                    for (name, w_sb, b_sb, dst, ncols) in (
                        ("k", wk_sb, bk_sb, kT, cw),
                        ("q", wq_sb, bq_sb, qT, qw_c),
                    ):
                        if ncols == 0:
                            continue
                        for m in range(HL):
                            ps = ps_proj.tile([128, ncols], F32, tag="proj")
                            for j in range(J):
                                nc.tensor.matmul(
                                    ps,
                                    lhsT=w_sb[:, j, m * 128:(m + 1) * 128],
                                    rhs=zT[ci][:, j, :ncols],
                                    start=(j == 0), stop=(j == J - 1),
                                )
                            nc.vector.tensor_scalar_add(
                                out=dst[:, m, c0:c0 + ncols], in0=ps,
                                scalar1=b_sb[:, m:m + 1],
                            )
                    # V projection for this chunk's token tiles (2-head chunks)
                    for i in range(tlo, thi):
                        for hh in range(HL // 2):
                            ps = ps_proj.tile([128, 192], F32, tag="proj")
                            for j in range(J):
                                nc.tensor.matmul(
                                    ps,
                                    lhsT=zt_slice(j, i * 128, 128),
                                    rhs=wv_sb[:, j, hh * 192:(hh + 1) * 192],
                                    start=(j == 0), stop=(j == J - 1),
                                )
                            nc.vector.tensor_tensor(
                                out=v_sb[:, i, 2 * hh:2 * hh + 2, 0:96],
                                in0=ps.rearrange("p (h c) -> p h c", c=96),
                                in1=bv_sb[:, hh * 192:(hh + 1) * 192].rearrange(
                                    "p (h c) -> p h c", c=96),
                                op=mybir.AluOpType.add,
                            )

                # overwrite kT row 96 of every head with the key-pad mask row
                km_bcast = bass.AP(
                    tensor=km_ext.tensor, offset=km_ext.offset,
                    ap=[[0, 1], [0, HL], [1, NA]],
                )
                nc.gpsimd.dma_start(out=kT[96:97, :, :], in_=km_bcast)

            # ---- attention over units (head, query-half), software-pipelined
            # by one unit: av matmuls of unit u-1 are emitted between the
            # scores of unit u so the (in-order) TensorE always has ready work
            # while ScalarE's exp stream — the phase bottleneck — runs back to
            # back.
            units = [(h, qs) for h in range(HL) for qs in QSPLIT]

            with (
                tc.tile_pool(name="ps_s", bufs=2, space="PSUM") as ps_s,
                tc.tile_pool(name="ps_av", bufs=2, space="PSUM") as ps_av,
            ):
                def scores_exp(u, tk):
                    h, (q0, qw) = u
                    s_ps = ps_s.tile([128, 576], F32, tag="s")
                    for (c0, cw) in _chunks(qw, 512):
                        nc.tensor.matmul(
                            s_ps[:, c0:c0 + cw],
                            lhsT=kT[:, h, tk * 128:(tk + 1) * 128],
                            rhs=qT[:, h, q0 + c0:q0 + c0 + cw],
                            start=True, stop=True,
                        )
                    probs = probs_pool.tile([128, 576], BF, tag="p")
                    nc.scalar.activation(
                        out=probs[:, :qw], in_=s_ps[:, :qw],
                        func=mybir.ActivationFunctionType.Exp,
                        scale=float(SCALE),
                    )
                    return probs

                def emit_av(u, tk, probs, av_ps):
                    h, (q0, qw) = u
                    for (c0, cw) in _chunks(qw, 512):
                        nc.tensor.matmul(
                            av_ps[:, c0:c0 + cw],
                            lhsT=v_sb[:, tk, h, :],
                            rhs=probs[:, c0:c0 + cw],
                            start=(tk == 0), stop=(tk == T - 1),
                            skip_group_check=True,
                        )

                def tail(u, av_ps):
                    h, (q0, qw) = u
                    # normalize: avn = avT[0:96] * (1 / avT[96]) broadcast.
                    d_sb = work.tile([1, qw], F32, tag="dsb")
                    nc.vector.tensor_copy(out=d_sb, in_=av_ps[96:97, :qw])
                    rec32 = work.tile([1, qw], F32, tag="rec32")
                    nc.vector.reciprocal_approx_fast(out=rec32, in_=d_sb)
                    recip_bf = work.tile([1, qw], BF, tag="recipbf")
                    nc.vector.tensor_copy(out=recip_bf, in_=rec32)
                    bc_sb = work.tile([96, qw], BF, tag="bc")
                    nc.gpsimd.partition_broadcast(out_ap=bc_sb, in_ap=recip_bf)
                    nc.vector.tensor_tensor(
                        out=avn[0:96, h, q0:q0 + qw],
                        in0=av_ps[0:96, :qw], in1=bc_sb,
                        op=mybir.AluOpType.mult,
                    )

                prev_probs = None
                prev_av = None
                prev_u = None
                for u in units:
                    cur_probs = [scores_exp(u, 0), scores_exp(u, 1)]
                    cur_av = ps_av.tile([128, 576], F32, tag="av")
                    if prev_probs is None:
                        for tk in range(2, T):
                            cur_probs.append(scores_exp(u, tk))
                    else:
                        k_av = 0
                        for tk in range(2, T):
                            emit_av(prev_u, k_av, prev_probs[k_av], prev_av)
                            k_av += 1
                            cur_probs.append(scores_exp(u, tk))
                        while k_av < T:
                            emit_av(prev_u, k_av, prev_probs[k_av], prev_av)
                            k_av += 1
                        tail(prev_u, prev_av)
                    prev_probs, prev_av, prev_u = cur_probs, cur_av, u
                for tk in range(T):
                    emit_av(prev_u, tk, prev_probs[tk], prev_av)
                tail(prev_u, prev_av)

            # ---- partial out projection (sum over local heads; host adds
            # the pair partials + bias)
            with tc.tile_pool(name="ps_o", bufs=2, space="PSUM") as ps_o:
                for m in range(J):
                    for (c0, cw) in _chunks(QC, 512):
                        o_ps = ps_o.tile([128, cw], F32, tag="o")
                        for h in range(HL):
                            nc.tensor.matmul(
                                o_ps,
                                lhsT=wo_sb[:, h, m * 128:(m + 1) * 128],
                                rhs=avn[:, h, c0:c0 + cw],
                                start=(h == 0), stop=(h == HL - 1),
                            )
                        o_sb = work.tile([128, cw], BF, tag="osb")
                        nc.vector.tensor_copy(out=o_sb, in_=o_ps)
                        nc.sync.dma_start(
                            out=out_ext[m * 128:(m + 1) * 128, c0:c0 + cw], in_=o_sb,
                        )

    nc.compile()
    return nc


_CACHE = {}


def _prep_weights(ln_g, ln_b, Wq, bq, Wk, bk, Wv, bv, Wo, bo):
    """Per-head-group device weight layouts. Returns [group0, group1]."""
    def pad_head_T(W, hg):
        # (W * g).T for heads of the group, padded 96 -> 128 cols, then
        # SBUF layout (128, J, HL*128): [p, j, e] = WT[j*128+p, e]
        WT = (W * ln_g[None, :]).T.astype(np.float32)
        WT = WT.reshape(D, H, 96)[:, hg * HL:(hg + 1) * HL, :]
        Wp = np.zeros((D, HL, 128), np.float32)
        Wp[:, :, :96] = WT
        Wp = Wp.reshape(J, 128, HL * 128).transpose(1, 0, 2)
        return np.ascontiguousarray(Wp.reshape(128, J * HL * 128)).astype(BF16)

    def plain_T(W, hg):
        WT = (W * ln_g[None, :]).T.astype(np.float32)
        WT = WT.reshape(D, H, 96)[:, hg * HL:(hg + 1) * HL, :].reshape(D, HL * 96)
        Wp = WT.reshape(J, 128, HL * 96).transpose(1, 0, 2)
        return np.ascontiguousarray(Wp.reshape(128, J * HL * 96)).astype(BF16)

    def pad_bias(bb, hg, ones_row):
        bp = np.zeros((HL, 128), np.float32)
        bp[:, :96] = bb.reshape(H, 96)[hg * HL:(hg + 1) * HL]
        if ones_row:
            bp[:, 96] = 1.0
        return np.ascontiguousarray(bp.T).astype(np.float32)  # (128, HL)

    def pad_wo(hg):
        w = np.zeros((128, HL, D), np.float32)
        w[:96] = Wo.T.reshape(H, 96, D)[hg * HL:(hg + 1) * HL].transpose(1, 0, 2)
        return np.ascontiguousarray(w.reshape(128, HL * D)).astype(BF16)

    bbq = Wq @ ln_b + bq
    bbk = Wk @ ln_b + bk
    bbv = Wv @ ln_b + bv
    return [{
        "wq": pad_head_T(Wq, hg),
        "wk": pad_head_T(Wk, hg),
        "wv": plain_T(Wv, hg),
        "wo": pad_wo(hg),
        "bq": pad_bias(bbq, hg, True),
        "bk": pad_bias(bbk, hg, False),
        "bv": np.ascontiguousarray(
            bbv.reshape(H, 96)[hg * HL:(hg + 1) * HL].reshape(-1)
        ).astype(np.float32),
    } for hg in range(2)]


def _make_in_maps(hidden_states, idx, NA, wmaps):
    in_maps = []
    for c in range(8):
        b, hg = c // 2, c % 2
        nb = len(idx[b])
        xg = np.zeros((NA, D), np.float32)
        xg[:nb] = hidden_states[b][idx[b]]
        km = np.zeros((NA,), np.float32)
        km[nb:] = MASK_NEG
        in_maps.append({
            "x": xg.astype(BF16),
            "km": km.reshape(1, NA).astype(BF16),
            **wmaps[hg],
        })
    return in_maps


def kernel(hidden_states, anchor_mask, ln_g, ln_b,
           Wq, bq, Wk, bk, Wv, bv, Wo, bo):
    hidden_states = np.asarray(hidden_states, dtype=np.float32)
    anchor_mask = np.asarray(anchor_mask).astype(bool)
    args = [np.asarray(a, dtype=np.float32)
            for a in (ln_g, ln_b, Wq, bq, Wk, bk, Wv, bv, Wo, bo)]
    bo_f = args[-1]

    idx = [np.nonzero(anchor_mask[b])[0] for b in range(B)]
    max_nb = max(len(i) for i in idx)
    NA = max(256, ((max_nb + 127) // 128) * 128)
    QC = max(128, ((max_nb + 63) // 64) * 64)

    if (NA, QC) not in _CACHE:
        _CACHE[(NA, QC)] = build(NA, QC)
    nc = _CACHE[(NA, QC)]

    wmaps = _prep_weights(*args)
    in_maps = _make_in_maps(hidden_states, idx, NA, wmaps)

    res = run_bass_kernel_spmd(nc, in_maps, core_ids=list(range(8)))

    out = np.zeros((B, S, D), np.float32)
    for b in range(B):
        nb = len(idx[b])
        oT = (res.results[2 * b]["out"].astype(np.float32)
              + res.results[2 * b + 1]["out"].astype(np.float32))
        out[b, idx[b]] = oT.T[:nb] + bo_f[None, :]
    return out
